# revision 12
# baseline (speedup 1.0000x reference)
"""CoupledLSTM Trainium2 kernel.

Problem: S=512, B=64, I=H=512 coupled-gate LSTM (f = 1-i), fp32 reference.

Strategy (8 NeuronCores, data-parallel over batch, 8 batch rows per core):
  - All device-side tensors keep hidden on the partition dim ("transposed"
    layout); the host does every layout transpose in numpy for free.
  - Phase A: xg[g] = x @ W_x[g].T + b[g] for all (t, b) as big matmuls
    (fp16 in, fp32 accumulate), kept SBUF-resident in fp16.
  - Phase B: 512 sequential steps; per step 48 [128x128]@[128x8] fp16
    matmuls (weight-port bound), fp32 elementwise on [128, 32] tiles.
Gate order everywhere: (c, i, o) so the c-gate chain overlaps i/o matmuls.
"""

import numpy as np

S, B, I, H = 512, 64, 512, 512
NCORES = 8
BL = B // NCORES  # 8 local batch rows
QI = I // 128  # 4 input chunks
QH = H // 128  # 4 hidden chunks
NB = (S * BL) // 512  # 8 phase-A N-blocks of 512 (t,b) columns
STEPS = S

_CACHE = {}


def _build_nc(steps=STEPS):
    import concourse.bacc as bacc
    import concourse.mybir as mybir
    import concourse.tile as tile

    f32 = mybir.dt.float32
    f16 = mybir.dt.float16
    AF = mybir.ActivationFunctionType

    nc = bacc.Bacc("TRN2", target_bir_lowering=False, debug=False, num_devices=NCORES)

    xT_d = nc.dram_tensor("xT", [I, S * BL], f16, kind="ExternalInput").ap()
    WxT_d = nc.dram_tensor("WxT", [3 * QH * QI * 128, 128], f16, kind="ExternalInput").ap()
    WhT_d = nc.dram_tensor("WhT", [3 * QH * QH * 128, 128], f16, kind="ExternalInput").ap()
    bias_d = nc.dram_tensor("bias", [128, 3 * QH], f32, kind="ExternalInput").ap()
    ident_d = nc.dram_tensor("ident", [128, 128], f16, kind="ExternalInput").ap()
    outT_d = nc.dram_tensor("outT", [S, 128, QH * BL], f16, kind="ExternalOutput").ap()
    cT_d = nc.dram_tensor("cT", [128, QH * BL], f32, kind="ExternalOutput").ap()

    GW = QH * BL  # 32: per-gate slab width (q, b)
    with tile.TileContext(nc) as tc:
        with tc.tile_pool(name="persist", bufs=1) as persist:
            # xg_all[p, gq, nb, ti, b]
            xg_all = persist.tile([128, 12, NB, 512 // BL, BL], f16)
            WhT = persist.tile([128, 3 * QH * QH * 128], f16)
            bias = persist.tile([128, 3 * QH], f32)
            nc.sync.dma_start(bias[:], bias_d[:])
            ident = persist.tile([128, 128], f16)
            nc.sync.dma_start(ident[:], ident_d[:])
            # WhT dram rows: tile index tt=(g,qo,qi) at rows [tt*128,(tt+1)*128);
            # SBUF wants [p_in, tt*128 + p_out].
            WhT_src = WhT_d.rearrange("(t p) m -> p t m", p=128)
            for i4 in range(4):
                nc.sync.dma_start(
                    WhT[:, i4 * 12 * 128 : (i4 + 1) * 12 * 128],
                    WhT_src[:, i4 * 12 : (i4 + 1) * 12, :],
                )

            # ---------------- Phase A ----------------
            with tc.tile_pool(name="phasea", bufs=1) as pa, tc.tile_pool(
                name="pa_psum", bufs=1, space="PSUM"
            ) as pap:
                WxT = pa.tile([128, 3 * QH * QI * 128], f16)
                WxT_src = WxT_d.rearrange("(t p) m -> p t m", p=128)
                for i4 in range(4):
                    nc.sync.dma_start(
                        WxT[:, i4 * 12 * 128 : (i4 + 1) * 12 * 128],
                        WxT_src[:, i4 * 12 : (i4 + 1) * 12, :],
                    )
                xts = []
                for qi in range(QI):
                    xt = pa.tile([128, S * BL], f16, tag=f"xt{qi}")
                    nc.sync.dma_start(xt[:], xT_d[qi * 128 : (qi + 1) * 128, :])
                    xts.append(xt)

                for g in range(3):
                    for qo in range(QH):
                        gq = g * QH + qo
                        for nb in range(NB):
                            ps = pap.tile([128, 512], f32, tag=f"ps{nb}")
                            for qi in range(QI):
                                tt = (g * QH + qo) * QI + qi
                                nc.tensor.matmul(
                                    ps[:],
                                    WxT[:, tt * 128 : (tt + 1) * 128],
                                    xts[qi][:, nb * 512 : (nb + 1) * 512],
                                    start=(qi == 0),
                                    stop=(qi == QI - 1),
                                )
                            nc.vector.tensor_scalar_add(
                                out=xg_all[:, gq, nb, :, :],
                                in0=ps[:],
                                scalar1=bias[:, gq : gq + 1],
                            )

            # ---------------- Phase B ----------------
            with tc.tile_pool(name="state", bufs=2) as st, tc.tile_pool(
                name="work", bufs=3
            ) as wk, tc.tile_pool(name="b_psum", bufs=2, space="PSUM") as pbp:
                h16 = st.tile([128, GW], f16, tag="h16")
                nc.vector.memset(h16[:], 0.0)
                c_st = st.tile([128, GW], f32, tag="c")
                nc.vector.memset(c_st[:], 0.0)

                def gate_mm(g, out_ap, h_prev, xg_slab, first, last):
                    # out = W_hg @ h + xg_g (identity matmul adds xg into psum).
                    # One PSUM bank allows a single start/stop accumulation
                    # group: start only on the bank's first matmul, stop on
                    # its last.
                    for qo in range(QH):
                        for qi in range(QH):
                            tt = (g * QH + qo) * QH + qi
                            nc.tensor.matmul(
                                out_ap[:, qo * BL : (qo + 1) * BL],
                                WhT[:, tt * 128 : (tt + 1) * 128],
                                h_prev[:, qi * BL : (qi + 1) * BL],
                                start=(first and qo == 0 and qi == 0),
                                stop=False,
                            )
                    nc.tensor.matmul(out_ap[:], ident[:], xg_slab, start=False, stop=last)

                for t in range(steps):
                    nb, ti = divmod(t, 512 // BL)

                    ps_c = pbp.tile([128, GW], f32, tag="ps_c")
                    gate_mm(0, ps_c[:], h16, xg_all[:, 0:QH, nb, ti, :], True, True)
                    ps_io = pbp.tile([128, 2 * GW], f32, tag="ps_io")
                    gate_mm(
                        1, ps_io[:, :GW], h16, xg_all[:, QH : 2 * QH, nb, ti, :], True, False
                    )
                    gate_mm(
                        2, ps_io[:, GW:], h16, xg_all[:, 2 * QH :, nb, ti, :], False, True
                    )

                    chat = wk.tile([128, GW], f32, tag="chat")
                    nc.scalar.activation(chat[:], ps_c[:], AF.Tanh)
                    d_t = wk.tile([128, GW], f32, tag="d")
                    nc.vector.tensor_sub(d_t[:], chat[:], c_st[:])

                    io_t = wk.tile([128, 2 * GW], f32, tag="io")
                    nc.scalar.activation(io_t[:], ps_io[:], AF.Sigmoid)
                    e_t = wk.tile([128, GW], f32, tag="e")
                    nc.vector.tensor_mul(e_t[:], io_t[:, :GW], d_t[:])
                    c_new = st.tile([128, GW], f32, tag="c")
                    nc.vector.tensor_add(c_new[:], c_st[:], e_t[:])
                    th = wk.tile([128, GW], f32, tag="th")
                    nc.scalar.activation(th[:], c_new[:], AF.Tanh)

                    h_new = st.tile([128, GW], f16, tag="h16")
                    nc.vector.tensor_mul(h_new[:], io_t[:, GW:], th[:])
                    nc.sync.dma_start(outT_d[t], h_new[:])

                    h16 = h_new
                    c_st = c_new

                nc.sync.dma_start(cT_d[:], c_st[:])

    nc.finalize()
    return nc


def _prep_weights(W_list):
    # W [H, K] -> tiles [(g qo qi) p_in, p_out]
    out = []
    for W in W_list:
        Wt = np.asarray(W, np.float32).reshape(QH, 128, -1, 128)  # qo p_out qi p_in
        out.append(np.transpose(Wt, (0, 2, 3, 1)))  # qo qi p_in p_out
    arr = np.stack(out, 0)  # g qo qi p_in p_out
    return np.ascontiguousarray(arr.reshape(-1, 128)).astype(np.float16)


def kernel(x, W_xi, W_hi, b_i, W_xc, W_hc, b_c, W_xo, W_ho, b_o):
    x = np.asarray(x, np.float32)
    # gate order (c, i, o)
    WxT = _prep_weights([W_xc, W_xi, W_xo])
    WhT = _prep_weights([W_hc, W_hi, W_ho])
    bias = np.stack(
        [np.asarray(b, np.float32).reshape(QH, 128).T for b in (b_c, b_i, b_o)], 1
    ).reshape(128, 3 * QH)

    if "nc" not in _CACHE:
        _CACHE["nc"] = _build_nc()
    nc = _CACHE["nc"]

    in_maps = []
    for c in range(NCORES):
        xs = x[:, c * BL : (c + 1) * BL, :]  # [S, BL, I]
        xT = np.ascontiguousarray(np.transpose(xs, (2, 0, 1)).reshape(I, S * BL))
        in_maps.append(
            {
                "xT": xT.astype(np.float16),
                "WxT": WxT,
                "WhT": WhT,
                "bias": bias,
                "ident": np.eye(128, dtype=np.float16),
            }
        )

    from concourse.bass_utils import run_bass_kernel_spmd

    _CACHE["in_maps"] = in_maps
    res = run_bass_kernel_spmd(nc, in_maps, list(range(NCORES)))

    output = np.empty((S, B, H), np.float32)
    c_fin = np.empty((B, H), np.float32)
    for c in range(NCORES):
        oT = res.results[c]["outT"].astype(np.float32)  # [S, 128, QH*BL]
        output[:, c * BL : (c + 1) * BL, :] = (
            oT.reshape(S, 128, QH, BL).transpose(0, 3, 2, 1).reshape(S, BL, H)
        )
        cT = res.results[c]["cT"]
        c_fin[c * BL : (c + 1) * BL] = (
            cT.reshape(128, QH, BL).transpose(2, 1, 0).reshape(BL, H)
        )
    h_fin = np.ascontiguousarray(output[-1])
    return output, h_fin, c_fin


# revision 14
# speedup vs baseline: 1.2621x; 1.2621x over previous
"""CoupledLSTM Trainium2 kernel.

Problem: S=512, B=64, I=H=512 coupled-gate LSTM (f = 1-i), fp32 reference.

Strategy (8 NeuronCores, data-parallel over batch, 8 batch rows per core):
  - All device-side tensors keep hidden on the partition dim ("transposed"
    layout); the host does every layout transpose in numpy for free.
  - Phase A: xg[g] = x @ W_x[g].T + b[g] for all (t, b) as big matmuls
    (fp16 in, fp32 accumulate), kept SBUF-resident in fp16.
  - Phase B: 512 sequential steps; per step 48 [128x128]@[128x8] fp16
    matmuls (weight-port bound), fp32 elementwise on [128, 32] tiles.
Gate order everywhere: (c, i, o) so the c-gate chain overlaps i/o matmuls.
"""

import numpy as np

S, B, I, H = 512, 64, 512, 512
NCORES = 8
BL = B // NCORES  # 8 local batch rows
QI = I // 128  # 4 input chunks
QH = H // 128  # 4 hidden chunks
NB = (S * BL) // 512  # 8 phase-A N-blocks of 512 (t,b) columns
STEPS = S

_CACHE = {}


def _build_nc(steps=STEPS):
    import concourse.bacc as bacc
    import concourse.mybir as mybir
    import concourse.tile as tile

    f32 = mybir.dt.float32
    f16 = mybir.dt.float16
    AF = mybir.ActivationFunctionType

    nc = bacc.Bacc("TRN2", target_bir_lowering=False, debug=False, num_devices=NCORES)

    xT_d = nc.dram_tensor("xT", [I, S * BL], f16, kind="ExternalInput").ap()
    WxT_d = nc.dram_tensor("WxT", [3 * QH * QI * 128, 128], f16, kind="ExternalInput").ap()
    WhT_d = nc.dram_tensor("WhT", [3 * QH * QH * 128, 128], f16, kind="ExternalInput").ap()
    bias_d = nc.dram_tensor("bias", [128, 3 * QH], f32, kind="ExternalInput").ap()
    ident_d = nc.dram_tensor("ident", [128, 128], f16, kind="ExternalInput").ap()
    outT_d = nc.dram_tensor("outT", [S, 128, QH * BL], f16, kind="ExternalOutput").ap()
    cT_d = nc.dram_tensor("cT", [128, QH * BL], f32, kind="ExternalOutput").ap()

    GW = QH * BL  # 32: per-gate slab width (q, b)
    with tile.TileContext(nc) as tc:
        with tc.tile_pool(name="persist", bufs=1) as persist:
            # xg_all[p, gq, nb, ti, b]
            xg_all = persist.tile([128, 12, NB, 512 // BL, BL], f16)
            WhT = persist.tile([128, 3 * QH * QH * 128], f16)
            bias = persist.tile([128, 3 * QH], f32)
            nc.sync.dma_start(bias[:], bias_d[:])
            ident = persist.tile([128, 128], f16)
            nc.sync.dma_start(ident[:], ident_d[:])
            # WhT dram rows: tile index tt=(g,qo,qi) at rows [tt*128,(tt+1)*128);
            # SBUF wants [p_in, tt*128 + p_out].
            WhT_src = WhT_d.rearrange("(t p) m -> p t m", p=128)
            for i4 in range(4):
                nc.sync.dma_start(
                    WhT[:, i4 * 12 * 128 : (i4 + 1) * 12 * 128],
                    WhT_src[:, i4 * 12 : (i4 + 1) * 12, :],
                )

            # ---------------- Phase A ----------------
            with tc.tile_pool(name="phasea", bufs=1) as pa, tc.tile_pool(
                name="pa_psum", bufs=1, space="PSUM"
            ) as pap:
                WxT = pa.tile([128, 3 * QH * QI * 128], f16)
                WxT_src = WxT_d.rearrange("(t p) m -> p t m", p=128)
                for i4 in range(4):
                    nc.sync.dma_start(
                        WxT[:, i4 * 12 * 128 : (i4 + 1) * 12 * 128],
                        WxT_src[:, i4 * 12 : (i4 + 1) * 12, :],
                    )
                xts = []
                for qi in range(QI):
                    xt = pa.tile([128, S * BL], f16, tag=f"xt{qi}")
                    nc.sync.dma_start(xt[:], xT_d[qi * 128 : (qi + 1) * 128, :])
                    xts.append(xt)

                for g in range(3):
                    for qo in range(QH):
                        gq = g * QH + qo
                        for nb in range(NB):
                            ps = pap.tile([128, 512], f32, tag=f"ps{nb}")
                            for qi in range(QI):
                                tt = (g * QH + qo) * QI + qi
                                nc.tensor.matmul(
                                    ps[:],
                                    WxT[:, tt * 128 : (tt + 1) * 128],
                                    xts[qi][:, nb * 512 : (nb + 1) * 512],
                                    start=(qi == 0),
                                    stop=(qi == QI - 1),
                                )
                            nc.vector.tensor_scalar_add(
                                out=xg_all[:, gq, nb, :, :],
                                in0=ps[:],
                                scalar1=bias[:, gq : gq + 1],
                            )

            # ---------------- Phase B ----------------
            with tc.tile_pool(name="state", bufs=2) as st, tc.tile_pool(
                name="work", bufs=3
            ) as wk, tc.tile_pool(name="b_psum", bufs=2, space="PSUM") as pbp:
                h16 = st.tile([128, GW], f16, tag="h16")
                nc.vector.memset(h16[:], 0.0)
                c_st = st.tile([128, GW], f32, tag="c")
                nc.vector.memset(c_st[:], 0.0)

                def gate_mm(g, out_ap, h_prev, xg_slab):
                    # out = W_hg @ h + xg_g (identity matmul adds xg into
                    # psum). One PSUM bank allows a single start/stop
                    # accumulation group: the identity matmul opens it
                    # (start=True zeroes the bank), the last W-matmul closes.
                    nc.tensor.matmul(out_ap[:], ident[:], xg_slab, start=True, stop=False)
                    for qo in range(QH):
                        for qi in range(QH):
                            tt = (g * QH + qo) * QH + qi
                            nc.tensor.matmul(
                                out_ap[:, qo * BL : (qo + 1) * BL],
                                WhT[:, tt * 128 : (tt + 1) * 128],
                                h_prev[:, qi * BL : (qi + 1) * BL],
                                start=False,
                                stop=(qo == QH - 1 and qi == QH - 1),
                            )

                for t in range(steps):
                    nb, ti = divmod(t, 512 // BL)

                    ps_c = pbp.tile([128, GW], f32, tag="ps_c")
                    gate_mm(0, ps_c[:], h16, xg_all[:, 0:QH, nb, ti, :])
                    ps_i = pbp.tile([128, GW], f32, tag="ps_i")
                    gate_mm(1, ps_i[:], h16, xg_all[:, QH : 2 * QH, nb, ti, :])
                    ps_o = pbp.tile([128, GW], f32, tag="ps_o")
                    gate_mm(2, ps_o[:], h16, xg_all[:, 2 * QH :, nb, ti, :])

                    chat = wk.tile([128, GW], f32, tag="chat")
                    nc.scalar.activation(chat[:], ps_c[:], AF.Tanh)
                    d_t = wk.tile([128, GW], f32, tag="d")
                    nc.vector.tensor_sub(d_t[:], chat[:], c_st[:])

                    i_t = wk.tile([128, GW], f32, tag="i")
                    nc.scalar.activation(i_t[:], ps_i[:], AF.Sigmoid)
                    e_t = wk.tile([128, GW], f32, tag="e")
                    nc.vector.tensor_mul(e_t[:], i_t[:], d_t[:])
                    c_new = st.tile([128, GW], f32, tag="c")
                    nc.vector.tensor_add(c_new[:], c_st[:], e_t[:])
                    th = wk.tile([128, GW], f32, tag="th")
                    nc.scalar.activation(th[:], c_new[:], AF.Tanh)

                    o_t = wk.tile([128, GW], f32, tag="o")
                    nc.scalar.activation(o_t[:], ps_o[:], AF.Sigmoid)
                    h_new = st.tile([128, GW], f16, tag="h16")
                    nc.vector.tensor_mul(h_new[:], o_t[:], th[:])
                    nc.sync.dma_start(outT_d[t], h_new[:])

                    h16 = h_new
                    c_st = c_new

                nc.sync.dma_start(cT_d[:], c_st[:])

    nc.finalize()
    return nc


def _prep_weights(W_list):
    # W [H, K] -> tiles [(g qo qi) p_in, p_out]
    out = []
    for W in W_list:
        Wt = np.asarray(W, np.float32).reshape(QH, 128, -1, 128)  # qo p_out qi p_in
        out.append(np.transpose(Wt, (0, 2, 3, 1)))  # qo qi p_in p_out
    arr = np.stack(out, 0)  # g qo qi p_in p_out
    return np.ascontiguousarray(arr.reshape(-1, 128)).astype(np.float16)


def kernel(x, W_xi, W_hi, b_i, W_xc, W_hc, b_c, W_xo, W_ho, b_o):
    x = np.asarray(x, np.float32)
    # gate order (c, i, o)
    WxT = _prep_weights([W_xc, W_xi, W_xo])
    WhT = _prep_weights([W_hc, W_hi, W_ho])
    bias = np.stack(
        [np.asarray(b, np.float32).reshape(QH, 128).T for b in (b_c, b_i, b_o)], 1
    ).reshape(128, 3 * QH)

    if "nc" not in _CACHE:
        _CACHE["nc"] = _build_nc()
    nc = _CACHE["nc"]

    in_maps = []
    for c in range(NCORES):
        xs = x[:, c * BL : (c + 1) * BL, :]  # [S, BL, I]
        xT = np.ascontiguousarray(np.transpose(xs, (2, 0, 1)).reshape(I, S * BL))
        in_maps.append(
            {
                "xT": xT.astype(np.float16),
                "WxT": WxT,
                "WhT": WhT,
                "bias": bias,
                "ident": np.eye(128, dtype=np.float16),
            }
        )

    from concourse.bass_utils import run_bass_kernel_spmd

    _CACHE["in_maps"] = in_maps
    res = run_bass_kernel_spmd(nc, in_maps, list(range(NCORES)))

    output = np.empty((S, B, H), np.float32)
    c_fin = np.empty((B, H), np.float32)
    for c in range(NCORES):
        oT = res.results[c]["outT"].astype(np.float32)  # [S, 128, QH*BL]
        output[:, c * BL : (c + 1) * BL, :] = (
            oT.reshape(S, 128, QH, BL).transpose(0, 3, 2, 1).reshape(S, BL, H)
        )
        cT = res.results[c]["cT"]
        c_fin[c * BL : (c + 1) * BL] = (
            cT.reshape(128, QH, BL).transpose(2, 1, 0).reshape(BL, H)
        )
    h_fin = np.ascontiguousarray(output[-1])
    return output, h_fin, c_fin


# revision 15
# speedup vs baseline: 1.2987x; 1.0291x over previous
"""CoupledLSTM Trainium2 kernel.

Problem: S=512, B=64, I=H=512 coupled-gate LSTM (f = 1-i), fp32 reference.

Strategy (8 NeuronCores, data-parallel over batch, 8 batch rows per core):
  - All device-side tensors keep hidden on the partition dim ("transposed"
    layout); the host does every layout transpose in numpy for free.
  - Phase A: xg[g] = x @ W_x[g].T + b[g] for all (t, b) as big matmuls
    (fp16 in, fp32 accumulate), kept SBUF-resident in fp16.
  - Phase B: 512 sequential steps; per step 48 [128x128]@[128x8] fp16
    matmuls (weight-port bound), fp32 elementwise on [128, 32] tiles.
Gate order everywhere: (c, i, o) so the c-gate chain overlaps i/o matmuls.
"""

import numpy as np

S, B, I, H = 512, 64, 512, 512
NCORES = 8
BL = B // NCORES  # 8 local batch rows
QI = I // 128  # 4 input chunks
QH = H // 128  # 4 hidden chunks
NB = (S * BL) // 512  # 8 phase-A N-blocks of 512 (t,b) columns
STEPS = S

_CACHE = {}


def _build_nc(steps=STEPS):
    import concourse.bacc as bacc
    import concourse.mybir as mybir
    import concourse.tile as tile

    f32 = mybir.dt.float32
    f16 = mybir.dt.float16
    AF = mybir.ActivationFunctionType

    nc = bacc.Bacc("TRN2", target_bir_lowering=False, debug=False, num_devices=NCORES)

    xT_d = nc.dram_tensor("xT", [I, S * BL], f16, kind="ExternalInput").ap()
    WxT_d = nc.dram_tensor("WxT", [3 * QH * QI * 128, 128], f16, kind="ExternalInput").ap()
    WhT_d = nc.dram_tensor("WhT", [3 * QH * QH * 128, 128], f16, kind="ExternalInput").ap()
    bias_d = nc.dram_tensor("bias", [128, 3 * QH], f32, kind="ExternalInput").ap()
    ident_d = nc.dram_tensor("ident", [128, 128], f16, kind="ExternalInput").ap()
    outT_d = nc.dram_tensor("outT", [S, 128, QH * BL], f16, kind="ExternalOutput").ap()
    cT_d = nc.dram_tensor("cT", [128, QH * BL], f32, kind="ExternalOutput").ap()

    GW = QH * BL  # 32: per-gate slab width (q, b)
    TIB = 512 // BL  # 64 steps per nb block
    with tile.TileContext(nc) as tc:
        with tc.tile_pool(name="persist", bufs=1) as persist, tc.tile_pool(
            name="state", bufs=2
        ) as st, tc.tile_pool(name="work", bufs=3) as wk, tc.tile_pool(
            name="b_psum", bufs=2, space="PSUM"
        ) as pbp, tc.tile_pool(name="pa_psum", bufs=2, space="PSUM") as pap:
            # xg_all[p, nb, gq, ti, b] fp16 (gate order c,i,o; gq = g*QH+qo)
            xg_all = persist.tile([128, NB, 12, TIB, BL], f16)
            WhT = persist.tile([128, 3 * QH * QH * 128], f16)
            WxT = persist.tile([128, 3 * QH * QI * 128], f16)
            bias = persist.tile([128, 3 * QH], f32)
            nc.sync.dma_start(bias[:], bias_d[:])
            ident = persist.tile([128, 128], f16)
            nc.sync.dma_start(ident[:], ident_d[:])
            # W dram rows: tile tt=(g,qo,qi) at rows [tt*128,(tt+1)*128);
            # SBUF wants [p_in, tt*128 + p_out].
            WhT_src = WhT_d.rearrange("(t p) m -> p t m", p=128)
            WxT_src = WxT_d.rearrange("(t p) m -> p t m", p=128)
            for i4 in range(4):
                nc.sync.dma_start(
                    WhT[:, i4 * 12 * 128 : (i4 + 1) * 12 * 128],
                    WhT_src[:, i4 * 12 : (i4 + 1) * 12, :],
                )
                nc.sync.dma_start(
                    WxT[:, i4 * 12 * 128 : (i4 + 1) * 12 * 128],
                    WxT_src[:, i4 * 12 : (i4 + 1) * 12, :],
                )
            xts = []
            for qi in range(QI):
                xt = persist.tile([128, S * BL], f16, tag=f"xt{qi}")
                nc.sync.dma_start(xt[:], xT_d[qi * 128 : (qi + 1) * 128, :])
                xts.append(xt)

            def pa_item(nb, gjc):
                # xg[nb, gjc] = W_x tile row @ x block + bias (one psum bank)
                ps = pap.tile([128, 512], f32, tag="pa")
                for qi in range(QI):
                    tt = gjc * QI + qi
                    nc.tensor.matmul(
                        ps[:],
                        WxT[:, tt * 128 : (tt + 1) * 128],
                        xts[qi][:, nb * 512 : (nb + 1) * 512],
                        start=(qi == 0),
                        stop=(qi == QI - 1),
                    )
                nc.vector.tensor_scalar_add(
                    out=xg_all[:, nb, gjc, :, :],
                    in0=ps[:],
                    scalar1=bias[:, gjc : gjc + 1],
                )

            # phase A upfront: first two nb blocks (steps 0..127)
            n_upfront_nb = min(2, (steps + TIB - 1) // TIB)
            for nb in range(n_upfront_nb):
                for gjc in range(12):
                    pa_item(nb, gjc)
            # remaining items dripped into the PE-idle tail windows of
            # early steps: nb's items spread over steps of block nb-2.
            drip = {}
            for nb in range(n_upfront_nb, NB):
                base = (nb - 2) * TIB + 2
                for k in range(12):
                    drip.setdefault(base + k * 5, []).append((nb, k))

            h16 = st.tile([128, GW], f16, tag="h16")
            nc.vector.memset(h16[:], 0.0)
            c_st = st.tile([128, GW], f32, tag="c")
            nc.vector.memset(c_st[:], 0.0)

            def gate_mm(g, out_ap, h_prev, xg_slab):
                # out = W_hg @ h + xg_g (identity matmul adds xg into
                # psum). One PSUM bank allows a single start/stop
                # accumulation group: the identity matmul opens it
                # (start=True zeroes the bank), the last W-matmul closes.
                nc.tensor.matmul(out_ap[:], ident[:], xg_slab, start=True, stop=False)
                for qo in range(QH):
                    for qi in range(QH):
                        tt = (g * QH + qo) * QH + qi
                        nc.tensor.matmul(
                            out_ap[:, qo * BL : (qo + 1) * BL],
                            WhT[:, tt * 128 : (tt + 1) * 128],
                            h_prev[:, qi * BL : (qi + 1) * BL],
                            start=False,
                            stop=(qo == QH - 1 and qi == QH - 1),
                        )

            for t in range(steps):
                nb, ti = divmod(t, TIB)

                ps_c = pbp.tile([128, GW], f32, tag="ps_c")
                gate_mm(0, ps_c[:], h16, xg_all[:, nb, 0:QH, ti, :])
                ps_i = pbp.tile([128, GW], f32, tag="ps_i")
                gate_mm(1, ps_i[:], h16, xg_all[:, nb, QH : 2 * QH, ti, :])
                ps_o = pbp.tile([128, GW], f32, tag="ps_o")
                gate_mm(2, ps_o[:], h16, xg_all[:, nb, 2 * QH :, ti, :])

                for nb2, gjc in drip.get(t, ()):
                    pa_item(nb2, gjc)

                chat = wk.tile([128, GW], f32, tag="chat")
                nc.scalar.activation(chat[:], ps_c[:], AF.Tanh)
                d_t = wk.tile([128, GW], f32, tag="d")
                nc.vector.tensor_sub(d_t[:], chat[:], c_st[:])

                i_t = wk.tile([128, GW], f32, tag="i")
                nc.scalar.activation(i_t[:], ps_i[:], AF.Sigmoid)
                e_t = wk.tile([128, GW], f32, tag="e")
                nc.vector.tensor_mul(e_t[:], i_t[:], d_t[:])
                c_new = st.tile([128, GW], f32, tag="c")
                nc.vector.tensor_add(c_new[:], c_st[:], e_t[:])
                th = wk.tile([128, GW], f32, tag="th")
                nc.scalar.activation(th[:], c_new[:], AF.Tanh)

                o_t = wk.tile([128, GW], f32, tag="o")
                nc.scalar.activation(o_t[:], ps_o[:], AF.Sigmoid)
                h_new = st.tile([128, GW], f16, tag="h16")
                nc.vector.tensor_mul(h_new[:], o_t[:], th[:])
                nc.sync.dma_start(outT_d[t], h_new[:])

                h16 = h_new
                c_st = c_new

            nc.sync.dma_start(cT_d[:], c_st[:])

    nc.finalize()
    return nc


def _prep_weights(W_list):
    # W [H, K] -> tiles [(g qo qi) p_in, p_out]
    out = []
    for W in W_list:
        Wt = np.asarray(W, np.float32).reshape(QH, 128, -1, 128)  # qo p_out qi p_in
        out.append(np.transpose(Wt, (0, 2, 3, 1)))  # qo qi p_in p_out
    arr = np.stack(out, 0)  # g qo qi p_in p_out
    return np.ascontiguousarray(arr.reshape(-1, 128)).astype(np.float16)


def kernel(x, W_xi, W_hi, b_i, W_xc, W_hc, b_c, W_xo, W_ho, b_o):
    x = np.asarray(x, np.float32)
    # gate order (c, i, o)
    WxT = _prep_weights([W_xc, W_xi, W_xo])
    WhT = _prep_weights([W_hc, W_hi, W_ho])
    bias = np.stack(
        [np.asarray(b, np.float32).reshape(QH, 128).T for b in (b_c, b_i, b_o)], 1
    ).reshape(128, 3 * QH)

    if "nc" not in _CACHE:
        _CACHE["nc"] = _build_nc()
    nc = _CACHE["nc"]

    in_maps = []
    for c in range(NCORES):
        xs = x[:, c * BL : (c + 1) * BL, :]  # [S, BL, I]
        xT = np.ascontiguousarray(np.transpose(xs, (2, 0, 1)).reshape(I, S * BL))
        in_maps.append(
            {
                "xT": xT.astype(np.float16),
                "WxT": WxT,
                "WhT": WhT,
                "bias": bias,
                "ident": np.eye(128, dtype=np.float16),
            }
        )

    from concourse.bass_utils import run_bass_kernel_spmd

    _CACHE["in_maps"] = in_maps
    res = run_bass_kernel_spmd(nc, in_maps, list(range(NCORES)))

    output = np.empty((S, B, H), np.float32)
    c_fin = np.empty((B, H), np.float32)
    for c in range(NCORES):
        oT = res.results[c]["outT"].astype(np.float32)  # [S, 128, QH*BL]
        output[:, c * BL : (c + 1) * BL, :] = (
            oT.reshape(S, 128, QH, BL).transpose(0, 3, 2, 1).reshape(S, BL, H)
        )
        cT = res.results[c]["cT"]
        c_fin[c * BL : (c + 1) * BL] = (
            cT.reshape(128, QH, BL).transpose(2, 1, 0).reshape(BL, H)
        )
    h_fin = np.ascontiguousarray(output[-1])
    return output, h_fin, c_fin


# revision 19
# speedup vs baseline: 1.3037x; 1.0038x over previous
"""CoupledLSTM Trainium2 kernel.

Problem: S=512, B=64, I=H=512 coupled-gate LSTM (f = 1-i), fp32 reference.

Strategy (8 NeuronCores, data-parallel over batch, 8 batch rows per core):
  - All device-side tensors keep hidden on the partition dim ("transposed"
    layout); the host does every layout transpose in numpy for free.
  - Phase A: xg[g] = x @ W_x[g].T + b[g] for all (t, b) as big matmuls
    (fp16 in, fp32 accumulate), kept SBUF-resident in fp16.
  - Phase B: 512 sequential steps; per step 48 [128x128]@[128x8] fp16
    matmuls (weight-port bound), fp32 elementwise on [128, 32] tiles.
Gate order everywhere: (c, i, o) so the c-gate chain overlaps i/o matmuls.
"""

import numpy as np

S, B, I, H = 512, 64, 512, 512
NCORES = 8
BL = B // NCORES  # 8 local batch rows
QI = I // 128  # 4 input chunks
QH = H // 128  # 4 hidden chunks
NB = (S * BL) // 512  # 8 phase-A N-blocks of 512 (t,b) columns
STEPS = S

_CACHE = {}


def _build_nc(steps=STEPS):
    import concourse.bacc as bacc
    import concourse.mybir as mybir
    import concourse.tile as tile

    f32 = mybir.dt.float32
    f16 = mybir.dt.float16
    AF = mybir.ActivationFunctionType

    nc = bacc.Bacc("TRN2", target_bir_lowering=False, debug=False, num_devices=NCORES)

    xT_d = nc.dram_tensor("xT", [I, S * BL], f16, kind="ExternalInput").ap()
    WxT_d = nc.dram_tensor("WxT", [3 * QH * QI * 128, 128], f16, kind="ExternalInput").ap()
    WhT_d = nc.dram_tensor("WhT", [3 * QH * QH * 128, 128], f16, kind="ExternalInput").ap()
    bias_d = nc.dram_tensor("bias", [128, 3 * QH], f32, kind="ExternalInput").ap()
    ident_d = nc.dram_tensor("ident", [128, 128], f16, kind="ExternalInput").ap()
    outT_d = nc.dram_tensor("outT", [S, 128, QH * BL], f16, kind="ExternalOutput").ap()
    cT_d = nc.dram_tensor("cT", [128, QH * BL], f32, kind="ExternalOutput").ap()

    GW = QH * BL  # 32: per-gate slab width (q, b)
    TIB = 512 // BL  # 64 steps per nb block
    with tile.TileContext(nc) as tc:
        with tc.tile_pool(name="persist", bufs=1) as persist, tc.tile_pool(
            name="state", bufs=2
        ) as st, tc.tile_pool(name="work", bufs=3) as wk, tc.tile_pool(
            name="b_psum", bufs=2, space="PSUM"
        ) as pbp, tc.tile_pool(name="pa_psum", bufs=2, space="PSUM") as pap:
            # xg_all[p, nb, gq, ti, b] fp16 (gate order c,i,o; gq = g*QH+qo)
            xg_all = persist.tile([128, NB, 12, TIB, BL], f16)
            WhT = persist.tile([128, 3 * QH * QH * 128], f16)
            WxT = persist.tile([128, 3 * QH * QI * 128], f16)
            bias = persist.tile([128, 3 * QH], f32)
            nc.sync.dma_start(bias[:], bias_d[:])
            ident = persist.tile([128, 128], f16)
            nc.sync.dma_start(ident[:], ident_d[:])
            # W dram rows: tile tt=(g,qo,qi) at rows [tt*128,(tt+1)*128);
            # SBUF wants [p_in, tt*128 + p_out]. Phase-A inputs (WxT, x)
            # load first so projection matmuls start ASAP; WhT (needed at
            # step 0, ~30us later) trails on the gpsimd queue.
            WhT_src = WhT_d.rearrange("(t p) m -> p t m", p=128)
            WxT_src = WxT_d.rearrange("(t p) m -> p t m", p=128)
            for i4 in range(4):
                nc.sync.dma_start(
                    WxT[:, i4 * 12 * 128 : (i4 + 1) * 12 * 128],
                    WxT_src[:, i4 * 12 : (i4 + 1) * 12, :],
                )
            xts = []
            for qi in range(QI):
                xt = persist.tile([128, S * BL], f16, tag=f"xt{qi}")
                nc.sync.dma_start(xt[:], xT_d[qi * 128 : (qi + 1) * 128, :])
                xts.append(xt)
            for i4 in range(4):
                nc.gpsimd.dma_start(
                    WhT[:, i4 * 12 * 128 : (i4 + 1) * 12 * 128],
                    WhT_src[:, i4 * 12 : (i4 + 1) * 12, :],
                )

            def pa_item(nb, gjc):
                # xg[nb, gjc] = W_x tile row @ x block + bias (one psum bank)
                ps = pap.tile([128, 512], f32, tag="pa")
                for qi in range(QI):
                    tt = gjc * QI + qi
                    nc.tensor.matmul(
                        ps[:],
                        WxT[:, tt * 128 : (tt + 1) * 128],
                        xts[qi][:, nb * 512 : (nb + 1) * 512],
                        start=(qi == 0),
                        stop=(qi == QI - 1),
                    )
                nc.vector.tensor_scalar_add(
                    out=xg_all[:, nb, gjc, :, :],
                    in0=ps[:],
                    scalar1=bias[:, gjc : gjc + 1],
                )

            # phase A upfront: first nb block (steps 0..63); the rest is
            # dripped into the PE-idle tail windows one block ahead of use.
            n_upfront_nb = min(1, (steps + TIB - 1) // TIB)
            for nb in range(n_upfront_nb):
                for gjc in range(12):
                    pa_item(nb, gjc)
            drip = {}
            for nb in range(n_upfront_nb, NB):
                base = (nb - 1) * TIB + 2
                for k in range(12):
                    drip.setdefault(base + k * 5, []).append((nb, k))

            h16 = st.tile([128, GW], f16, tag="h16")
            nc.vector.memset(h16[:], 0.0)
            c_st = st.tile([128, GW], f32, tag="c")
            nc.vector.memset(c_st[:], 0.0)

            def gate_mm(g, out_ap, h_prev, xg_slab):
                # out = W_hg @ h + xg_g (identity matmul adds xg into
                # psum). One PSUM bank allows a single start/stop
                # accumulation group: the identity matmul opens it
                # (start=True zeroes the bank), the last W-matmul closes.
                nc.tensor.matmul(out_ap[:], ident[:], xg_slab, start=True, stop=False)
                for qo in range(QH):
                    for qi in range(QH):
                        tt = (g * QH + qo) * QH + qi
                        nc.tensor.matmul(
                            out_ap[:, qo * BL : (qo + 1) * BL],
                            WhT[:, tt * 128 : (tt + 1) * 128],
                            h_prev[:, qi * BL : (qi + 1) * BL],
                            start=False,
                            stop=(qo == QH - 1 and qi == QH - 1),
                        )

            for t in range(steps):
                nb, ti = divmod(t, TIB)

                ps_c = pbp.tile([128, GW], f32, tag="ps_c")
                gate_mm(0, ps_c[:], h16, xg_all[:, nb, 0:QH, ti, :])
                ps_i = pbp.tile([128, GW], f32, tag="ps_i")
                gate_mm(1, ps_i[:], h16, xg_all[:, nb, QH : 2 * QH, ti, :])
                ps_o = pbp.tile([128, GW], f32, tag="ps_o")
                gate_mm(2, ps_o[:], h16, xg_all[:, nb, 2 * QH :, ti, :])

                chat = wk.tile([128, GW], f32, tag="chat")
                nc.scalar.activation(chat[:], ps_c[:], AF.Tanh)
                d_t = wk.tile([128, GW], f32, tag="d")
                nc.vector.tensor_sub(d_t[:], chat[:], c_st[:])

                i_t = wk.tile([128, GW], f32, tag="i")
                nc.scalar.activation(i_t[:], ps_i[:], AF.Sigmoid)
                e_t = wk.tile([128, GW], f32, tag="e")
                nc.vector.tensor_mul(e_t[:], i_t[:], d_t[:])
                c_new = st.tile([128, GW], f32, tag="c")
                nc.vector.tensor_add(c_new[:], c_st[:], e_t[:])
                th = wk.tile([128, GW], f32, tag="th")
                nc.scalar.activation(th[:], c_new[:], AF.Tanh)

                o_t = wk.tile([128, GW], f32, tag="o")
                nc.scalar.activation(o_t[:], ps_o[:], AF.Sigmoid)
                h_new = st.tile([128, GW], f16, tag="h16")
                nc.vector.tensor_mul(h_new[:], o_t[:], th[:])
                nc.sync.dma_start(outT_d[t], h_new[:])

                for nb2, gjc in drip.get(t, ()):
                    pa_item(nb2, gjc)

                h16 = h_new
                c_st = c_new

            nc.sync.dma_start(cT_d[:], c_st[:])

    nc.finalize()
    return nc


def _prep_weights(W_list):
    # W [H, K] -> tiles [(g qo qi) p_in, p_out]
    out = []
    for W in W_list:
        Wt = np.asarray(W, np.float32).reshape(QH, 128, -1, 128)  # qo p_out qi p_in
        out.append(np.transpose(Wt, (0, 2, 3, 1)))  # qo qi p_in p_out
    arr = np.stack(out, 0)  # g qo qi p_in p_out
    return np.ascontiguousarray(arr.reshape(-1, 128)).astype(np.float16)


def kernel(x, W_xi, W_hi, b_i, W_xc, W_hc, b_c, W_xo, W_ho, b_o):
    x = np.asarray(x, np.float32)
    # gate order (c, i, o)
    WxT = _prep_weights([W_xc, W_xi, W_xo])
    WhT = _prep_weights([W_hc, W_hi, W_ho])
    bias = np.stack(
        [np.asarray(b, np.float32).reshape(QH, 128).T for b in (b_c, b_i, b_o)], 1
    ).reshape(128, 3 * QH)

    if "nc" not in _CACHE:
        _CACHE["nc"] = _build_nc()
    nc = _CACHE["nc"]

    in_maps = []
    for c in range(NCORES):
        xs = x[:, c * BL : (c + 1) * BL, :]  # [S, BL, I]
        xT = np.ascontiguousarray(np.transpose(xs, (2, 0, 1)).reshape(I, S * BL))
        in_maps.append(
            {
                "xT": xT.astype(np.float16),
                "WxT": WxT,
                "WhT": WhT,
                "bias": bias,
                "ident": np.eye(128, dtype=np.float16),
            }
        )

    from concourse.bass_utils import run_bass_kernel_spmd

    _CACHE["in_maps"] = in_maps
    res = run_bass_kernel_spmd(nc, in_maps, list(range(NCORES)))

    output = np.empty((S, B, H), np.float32)
    c_fin = np.empty((B, H), np.float32)
    for c in range(NCORES):
        oT = res.results[c]["outT"].astype(np.float32)  # [S, 128, QH*BL]
        output[:, c * BL : (c + 1) * BL, :] = (
            oT.reshape(S, 128, QH, BL).transpose(0, 3, 2, 1).reshape(S, BL, H)
        )
        cT = res.results[c]["cT"]
        c_fin[c * BL : (c + 1) * BL] = (
            cT.reshape(128, QH, BL).transpose(2, 1, 0).reshape(BL, H)
        )
    h_fin = np.ascontiguousarray(output[-1])
    return output, h_fin, c_fin


# revision 21
# speedup vs baseline: 1.3076x; 1.0030x over previous
"""CoupledLSTM Trainium2 kernel.

Problem: S=512, B=64, I=H=512 coupled-gate LSTM (f = 1-i), fp32 reference.

Strategy (8 NeuronCores, data-parallel over batch, 8 batch rows per core):
  - All device-side tensors keep hidden on the partition dim ("transposed"
    layout); the host does every layout transpose in numpy for free.
  - Phase A: xg[g] = x @ W_x[g].T + b[g] for all (t, b) as big matmuls
    (fp16 in, fp32 accumulate), kept SBUF-resident in fp16.
  - Phase B: 512 sequential steps; per step 48 [128x128]@[128x8] fp16
    matmuls (weight-port bound), fp32 elementwise on [128, 32] tiles.
Gate order everywhere: (c, i, o) so the c-gate chain overlaps i/o matmuls.
"""

import numpy as np

S, B, I, H = 512, 64, 512, 512
NCORES = 8
BL = B // NCORES  # 8 local batch rows
QI = I // 128  # 4 input chunks
QH = H // 128  # 4 hidden chunks
NB = (S * BL) // 512  # 8 phase-A N-blocks of 512 (t,b) columns
STEPS = S

_CACHE = {}


def _build_nc(steps=STEPS):
    import concourse.bacc as bacc
    import concourse.mybir as mybir
    import concourse.tile as tile

    f32 = mybir.dt.float32
    f16 = mybir.dt.float16
    AF = mybir.ActivationFunctionType

    nc = bacc.Bacc("TRN2", target_bir_lowering=False, debug=False, num_devices=NCORES)

    xT_d = nc.dram_tensor("xT", [I, S * BL], f16, kind="ExternalInput").ap()
    WxT_d = nc.dram_tensor("WxT", [3 * QH * QI * 128, 128], f16, kind="ExternalInput").ap()
    WhT_d = nc.dram_tensor("WhT", [3 * QH * QH * 128, 128], f16, kind="ExternalInput").ap()
    bias_d = nc.dram_tensor("bias", [128, 3 * QH], f32, kind="ExternalInput").ap()
    ident_d = nc.dram_tensor("ident", [128, 128], f16, kind="ExternalInput").ap()
    outT_d = nc.dram_tensor("outT", [S, 128, QH * BL], f16, kind="ExternalOutput").ap()
    cT_d = nc.dram_tensor("cT", [128, QH * BL], f32, kind="ExternalOutput").ap()

    GW = QH * BL  # 32: per-gate slab width (q, b)
    TIB = 512 // BL  # 64 steps per nb block
    with tile.TileContext(nc) as tc:
        with tc.tile_pool(name="persist", bufs=1) as persist, tc.tile_pool(
            name="state", bufs=2
        ) as st, tc.tile_pool(name="work", bufs=3) as wk, tc.tile_pool(
            name="b_psum", bufs=2, space="PSUM"
        ) as pbp, tc.tile_pool(name="pa_psum", bufs=2, space="PSUM") as pap:
            # xg_all[p, nb, gq, ti, b] fp16 (gate order c,i,o; gq = g*QH+qo)
            xg_all = persist.tile([128, NB, 12, TIB, BL], f16)
            WhT = persist.tile([128, 3 * QH * QH * 128], f16)
            WxT = persist.tile([128, 3 * QH * QI * 128], f16)
            bias = persist.tile([128, 3 * QH], f32)
            nc.sync.dma_start(bias[:], bias_d[:])
            ident = persist.tile([128, 128], f16)
            nc.sync.dma_start(ident[:], ident_d[:])
            # W dram rows: tile tt=(g,qo,qi) at rows [tt*128,(tt+1)*128);
            # SBUF wants [p_in, tt*128 + p_out]. Phase-A inputs (WxT, x)
            # load first so projection matmuls start ASAP; WhT (needed at
            # step 0, ~30us later) trails on the gpsimd queue.
            WhT_src = WhT_d.rearrange("(t p) m -> p t m", p=128)
            WxT_src = WxT_d.rearrange("(t p) m -> p t m", p=128)
            qs = [nc.sync, nc.gpsimd, nc.scalar]
            for i4 in range(4):
                qs[i4 % 2].dma_start(
                    WxT[:, i4 * 12 * 128 : (i4 + 1) * 12 * 128],
                    WxT_src[:, i4 * 12 : (i4 + 1) * 12, :],
                )
            xts = []
            for qi in range(QI):
                xt = persist.tile([128, S * BL], f16, tag=f"xt{qi}")
                qs[2 if qi % 2 else 0].dma_start(xt[:], xT_d[qi * 128 : (qi + 1) * 128, :])
                xts.append(xt)
            for i4 in range(4):
                qs[i4 % 2].dma_start(
                    WhT[:, i4 * 12 * 128 : (i4 + 1) * 12 * 128],
                    WhT_src[:, i4 * 12 : (i4 + 1) * 12, :],
                )

            def pa_item(nb, gjc):
                # xg[nb, gjc] = W_x tile row @ x block + bias (one psum bank)
                ps = pap.tile([128, 512], f32, tag="pa")
                for qi in range(QI):
                    tt = gjc * QI + qi
                    nc.tensor.matmul(
                        ps[:],
                        WxT[:, tt * 128 : (tt + 1) * 128],
                        xts[qi][:, nb * 512 : (nb + 1) * 512],
                        start=(qi == 0),
                        stop=(qi == QI - 1),
                    )
                nc.vector.tensor_scalar_add(
                    out=xg_all[:, nb, gjc, :, :],
                    in0=ps[:],
                    scalar1=bias[:, gjc : gjc + 1],
                )

            # phase A upfront: first nb block (steps 0..63); the rest is
            # dripped into the PE-idle tail windows one block ahead of use.
            n_upfront_nb = min(1, (steps + TIB - 1) // TIB)
            for nb in range(n_upfront_nb):
                for gjc in range(12):
                    pa_item(nb, gjc)
            drip = {}
            for nb in range(n_upfront_nb, NB):
                base = (nb - 1) * TIB + 2
                for k in range(12):
                    drip.setdefault(base + k * 5, []).append((nb, k))

            h16 = st.tile([128, GW], f16, tag="h16")
            nc.vector.memset(h16[:], 0.0)
            c_st = st.tile([128, GW], f32, tag="c")
            nc.vector.memset(c_st[:], 0.0)

            def gate_mm(g, out_ap, h_prev, xg_slab):
                # out = W_hg @ h + xg_g (identity matmul adds xg into
                # psum). One PSUM bank allows a single start/stop
                # accumulation group: the identity matmul opens it
                # (start=True zeroes the bank), the last W-matmul closes.
                nc.tensor.matmul(out_ap[:], ident[:], xg_slab, start=True, stop=False)
                for qo in range(QH):
                    for qi in range(QH):
                        tt = (g * QH + qo) * QH + qi
                        nc.tensor.matmul(
                            out_ap[:, qo * BL : (qo + 1) * BL],
                            WhT[:, tt * 128 : (tt + 1) * 128],
                            h_prev[:, qi * BL : (qi + 1) * BL],
                            start=False,
                            stop=(qo == QH - 1 and qi == QH - 1),
                        )

            for t in range(steps):
                nb, ti = divmod(t, TIB)

                ps_c = pbp.tile([128, GW], f32, tag="ps_c")
                gate_mm(0, ps_c[:], h16, xg_all[:, nb, 0:QH, ti, :])
                ps_i = pbp.tile([128, GW], f32, tag="ps_i")
                gate_mm(1, ps_i[:], h16, xg_all[:, nb, QH : 2 * QH, ti, :])
                ps_o = pbp.tile([128, GW], f32, tag="ps_o")
                gate_mm(2, ps_o[:], h16, xg_all[:, nb, 2 * QH :, ti, :])

                chat = wk.tile([128, GW], f32, tag="chat")
                nc.scalar.activation(chat[:], ps_c[:], AF.Tanh)
                d_t = wk.tile([128, GW], f32, tag="d")
                nc.vector.tensor_sub(d_t[:], chat[:], c_st[:])

                i_t = wk.tile([128, GW], f32, tag="i")
                nc.scalar.activation(i_t[:], ps_i[:], AF.Sigmoid)
                e_t = wk.tile([128, GW], f32, tag="e")
                nc.vector.tensor_mul(e_t[:], i_t[:], d_t[:])
                c_new = st.tile([128, GW], f32, tag="c")
                nc.vector.tensor_add(c_new[:], c_st[:], e_t[:])
                th = wk.tile([128, GW], f32, tag="th")
                nc.scalar.activation(th[:], c_new[:], AF.Tanh)

                o_t = wk.tile([128, GW], f32, tag="o")
                nc.scalar.activation(o_t[:], ps_o[:], AF.Sigmoid)
                h_new = st.tile([128, GW], f16, tag="h16")
                nc.vector.tensor_mul(h_new[:], o_t[:], th[:])
                nc.sync.dma_start(outT_d[t], h_new[:])

                for nb2, gjc in drip.get(t, ()):
                    pa_item(nb2, gjc)

                h16 = h_new
                c_st = c_new

            nc.sync.dma_start(cT_d[:], c_st[:])

    nc.finalize()
    return nc


def _prep_weights(W_list):
    # W [H, K] -> tiles [(g qo qi) p_in, p_out]
    out = []
    for W in W_list:
        Wt = np.asarray(W, np.float32).reshape(QH, 128, -1, 128)  # qo p_out qi p_in
        out.append(np.transpose(Wt, (0, 2, 3, 1)))  # qo qi p_in p_out
    arr = np.stack(out, 0)  # g qo qi p_in p_out
    return np.ascontiguousarray(arr.reshape(-1, 128)).astype(np.float16)


def kernel(x, W_xi, W_hi, b_i, W_xc, W_hc, b_c, W_xo, W_ho, b_o):
    x = np.asarray(x, np.float32)
    # gate order (c, i, o)
    WxT = _prep_weights([W_xc, W_xi, W_xo])
    WhT = _prep_weights([W_hc, W_hi, W_ho])
    bias = np.stack(
        [np.asarray(b, np.float32).reshape(QH, 128).T for b in (b_c, b_i, b_o)], 1
    ).reshape(128, 3 * QH)

    if "nc" not in _CACHE:
        _CACHE["nc"] = _build_nc()
    nc = _CACHE["nc"]

    in_maps = []
    for c in range(NCORES):
        xs = x[:, c * BL : (c + 1) * BL, :]  # [S, BL, I]
        xT = np.ascontiguousarray(np.transpose(xs, (2, 0, 1)).reshape(I, S * BL))
        in_maps.append(
            {
                "xT": xT.astype(np.float16),
                "WxT": WxT,
                "WhT": WhT,
                "bias": bias,
                "ident": np.eye(128, dtype=np.float16),
            }
        )

    from concourse.bass_utils import run_bass_kernel_spmd

    _CACHE["in_maps"] = in_maps
    res = run_bass_kernel_spmd(nc, in_maps, list(range(NCORES)))

    output = np.empty((S, B, H), np.float32)
    c_fin = np.empty((B, H), np.float32)
    for c in range(NCORES):
        oT = res.results[c]["outT"].astype(np.float32)  # [S, 128, QH*BL]
        output[:, c * BL : (c + 1) * BL, :] = (
            oT.reshape(S, 128, QH, BL).transpose(0, 3, 2, 1).reshape(S, BL, H)
        )
        cT = res.results[c]["cT"]
        c_fin[c * BL : (c + 1) * BL] = (
            cT.reshape(128, QH, BL).transpose(2, 1, 0).reshape(BL, H)
        )
    h_fin = np.ascontiguousarray(output[-1])
    return output, h_fin, c_fin
